# revision 48
# baseline (speedup 1.0000x reference)
"""Quantized-AlexNet forward on 8 trn2 NeuronCores.

Strategy (v2):
  - data-parallel conv stack: 8 images per core
  - conv1: host im2col in bf16 (padded to 3072 cols), bf16 matmul (1 cyc/row
    vs 4 for fp32), BN affine folded into weights/bias
  - conv2: fp32r matmuls with exact +-1 sign weights (unchanged from v1)
  - conv3..5: all-bf16 (sign weights exact in bf16); activations bf16;
    c5in aliases c3in buffers to save SBUF
  - maxpools: 2-pass DVE pool, bf16 where possible (2x DVE throughput)
  - FC: tensor-parallel on output dim; bf16 weights (+-1 exact) and
    activations; transposed h via PE; ONE AllGather per FC layer in
    [feat, img] layout; fw1 first half prefetched in persistent SBUF,
    second half + fw2/fw3 streamed under compute; per-half h AllGather
    issued inside conv5; DoReFa scales + biases in fp32 epilogues
"""

import os
import numpy as np
import ml_dtypes

NCORES = 8
B = 64
BPC = B // NCORES  # images per core

_PROG_CACHE = {}
LAST_EXEC_NS = None
LAST_RESULTS = None


def _build_program():
    import concourse.bass as bass
    import concourse.mybir as mybir
    import concourse.tile as tile
    from concourse import bacc
    from concourse.masks import make_identity

    F32 = mybir.dt.float32
    F32R = mybir.dt.float32r
    BF16 = mybir.dt.bfloat16
    AF = mybir.ActivationFunctionType

    def mkap(tile_ap, offset_elems, dims):
        part = tile_ap.ap[0]
        return bass.AP(
            tensor=tile_ap.tensor,
            offset=tile_ap.offset + offset_elems,
            ap=[list(part)] + [list(d) for d in dims],
        )

    def rawap(tile_ap, offset_elems, dims):
        return bass.AP(
            tensor=tile_ap.tensor,
            offset=tile_ap.offset + offset_elems,
            ap=[list(d) for d in dims],
        )

    nc = bacc.Bacc("TRN2", target_bir_lowering=False, debug=False,
                   num_devices=NCORES)

    def max3(out_ap, mk_in, step):
        nc.vector.tensor_max(out_ap, mk_in(0), mk_in(step))
        nc.vector.tensor_max(out_ap, out_ap, mk_in(2 * step))

    # ---- DRAM I/O ----
    xcol_d = nc.dram_tensor("xcol", [BPC, 3, 121, 3072], BF16, kind="ExternalInput").ap()
    w1c_d = nc.dram_tensor("w1c", [3, 121, 96], BF16, kind="ExternalInput").ap()
    sc_d = nc.dram_tensor("scall", [128, 24], F32, kind="ExternalInput").ap()
    w2c_d = nc.dram_tensor("w2c", [25, 96, 256], F32, kind="ExternalInput").ap()
    w3c_d = nc.dram_tensor("w3c", [3, 18, 128, 128], BF16, kind="ExternalInput").ap()
    w4c_d = nc.dram_tensor("w4c", [3, 27, 128, 128], BF16, kind="ExternalInput").ap()
    w5c_d = nc.dram_tensor("w5c", [2, 27, 128, 128], BF16, kind="ExternalInput").ap()
    fw1_d = nc.dram_tensor("fw1t", [9216, 512], BF16, kind="ExternalInput").ap()
    fb1_d = nc.dram_tensor("fb1r", [1, 512], F32, kind="ExternalInput").ap()
    fw2_d = nc.dram_tensor("fw2t", [4096, 512], BF16, kind="ExternalInput").ap()
    fb2_d = nc.dram_tensor("fb2r", [1, 512], F32, kind="ExternalInput").ap()
    fw3_d = nc.dram_tensor("fw3t", [4096, 126], BF16, kind="ExternalInput").ap()
    fb3_d = nc.dram_tensor("fb3r", [1, 126], F32, kind="ExternalInput").ap()
    out_d = nc.dram_tensor("out", [64, 126], F32, kind="ExternalOutput").ap()

    # sc_sb column pairs: 0:conv1, 1-2:conv2(m0,m1), 3-5:conv3, 6-8:conv4,
    # 9-10:conv5, 11: (Ef1, Ef2)
    SC1, SC2, SC3, SC4, SC5, SCF = 0, 1, 3, 6, 9, 11

    with tile.TileContext(nc) as tc:
        with tc.tile_pool(name="wp", bufs=1) as wp, \
             tc.tile_pool(name="dr", bufs=1, space="DRAM") as dr:

            # ---- persistent tiles ----
            w1sb = wp.tile([121, 3 * 96], BF16, name="w1sb")
            nc.sync.dma_start(
                out=w1sb.rearrange("p (c m) -> p c m", c=3),
                in_=bass.AP(tensor=w1c_d.tensor, offset=0,
                            ap=[[96, 121], [121 * 96, 3], [1, 96]]))
            sc_sb = wp.tile([128, 24], F32, name="sc_sb")
            nc.sync.dma_start(out=sc_sb, in_=sc_d)

            w2sb = wp.tile([96, 25 * 256], F32R, name="w2sb")
            nc.gpsimd.dma_start(
                out=w2sb.rearrange("p (s m) -> p s m", s=25),
                in_=bass.AP(tensor=w2c_d.tensor, offset=0,
                            ap=[[256, 96], [96 * 256, 25], [1, 256]]
                            ).bitcast(F32R))

            # fw1 first half [kt 0..35] prefetched persistently (3 DMAs)
            fw1a = wp.tile([128, 36 * 512], BF16, name="fw1a")
            for gi in range(3):
                nc.gpsimd.dma_start(
                    out=fw1a[:, gi * 12 * 512:(gi + 1) * 12 * 512]
                        .rearrange("p (i f) -> p i f", i=12),
                    in_=bass.AP(tensor=fw1_d.tensor,
                                offset=gi * 12 * 128 * 512,
                                ap=[[512, 128], [128 * 512, 12], [1, 512]]))

            fb1r = wp.tile([64, 512], F32, name="fb1r")
            nc.gpsimd.dma_start(out=fb1r, in_=bass.AP(
                tensor=fb1_d.tensor, offset=0, ap=[[0, 64], [1, 512]]))
            fb2r = wp.tile([64, 512], F32, name="fb2r")
            nc.gpsimd.dma_start(out=fb2r, in_=bass.AP(
                tensor=fb2_d.tensor, offset=0, ap=[[0, 64], [1, 512]]))
            fb3r = wp.tile([64, 126], F32, name="fb3r")
            nc.gpsimd.dma_start(out=fb3r, in_=bass.AP(
                tensor=fb3_d.tensor, offset=0, ap=[[0, 64], [1, 126]]))

            # DRAM scratch
            h_loc = dr.tile([BPC, 9216], BF16, name="h_loc")
            h_all = dr.tile([B, 9216], BF16, addr_space="Shared", name="h_all")
            y1loc = dr.tile([512, 64], BF16, name="y1loc")
            y1all = dr.tile([4096, 64], BF16, addr_space="Shared", name="y1all")
            y2loc = dr.tile([512, 64], BF16, name="y2loc")
            y2all = dr.tile([4096, 64], BF16, addr_space="Shared", name="y2all")

            # ============== conv stack ==============
            with tc.tile_pool(name="p345", bufs=1) as p345, \
                 tc.tile_pool(name="wq", bufs=2) as wq, \
                 tc.tile_pool(name="psA", bufs=6, space="PSUM") as psA:

                c3in = {}

                def pad_tile(tag):
                    t = p345.tile([128, 2 * 240], BF16, tag=tag, name=tag)
                    nc.gpsimd.memset(t, 0.0)
                    return t

                with tc.tile_pool(name="xc", bufs=6) as xc, \
                     tc.tile_pool(name="act", bufs=1) as act:

                    c2ins = {}

                    def conv1_pool1(i):
                        g, islot = divmod(i, 2)
                        c1out = act.tile([96, 3072], BF16, tag="c1out", bufs=1,
                                         name=f"c1out_{i}")
                        for k in range(6):
                            xt = xc.tile([121, 3, 512], BF16, tag="xc")
                            nc.sync.dma_start(
                                out=xt,
                                in_=bass.AP(tensor=xcol_d.tensor,
                                            offset=i * 3 * 121 * 3072 + k * 512,
                                            ap=[[3072, 121], [121 * 3072, 3],
                                                [1, 512]]))
                            pt = psA.tile([128, 512], F32, tag="psc1", bufs=2)
                            for c in range(3):
                                nc.tensor.matmul(
                                    pt[:96], w1sb[:, c * 96:(c + 1) * 96],
                                    xt[:, c, :], start=(c == 0), stop=(c == 2))
                            nc.scalar.activation(
                                c1out[:, k * 512:(k + 1) * 512], pt[:96],
                                AF.Relu, bias=sc_sb[:96, 2 * SC1 + 1:2 * SC1 + 2],
                                scale=1.0)

                        p1 = act.tile([96, 55 * 27], BF16, tag="p1", bufs=2,
                                      name=f"p1_{i}")
                        max3(mkap(p1, 0, [[27, 55], [1, 27]]),
                             lambda o: mkap(c1out, o, [[55, 55], [2, 27]]), 1)
                        if islot == 0:
                            c2in = act.tile([96, 2 * 31 * 32], F32R, tag="c2in",
                                            bufs=2, name=f"c2in_{g}")
                            nc.gpsimd.memset(c2in.bitcast(F32), 0.0)
                            c2ins[g] = c2in
                            c3in[g] = [pad_tile(f"c3in{kb}_{g}") for kb in range(2)]
                        c2in = c2ins[g]
                        c2wv = mkap(c2in, islot * 992 + 2 * 32 + 2,
                                    [[1, 27], [32, 27]])
                        ptmp = act.tile([96, 27 * 27], BF16, tag="ptmp", bufs=2)
                        nc.vector.tensor_max(
                            mkap(ptmp, 0, [[1, 27], [27, 27]]),
                            mkap(p1, 0, [[1, 27], [54, 27]]),
                            mkap(p1, 27, [[1, 27], [54, 27]]))
                        nc.vector.tensor_max(c2wv,
                                             mkap(ptmp, 0, [[1, 27], [27, 27]]),
                                             mkap(p1, 54, [[1, 27], [54, 27]]))

                    def conv2_pool2(g):
                        c2in = c2ins[g]
                        c2pt = {}
                        for m in range(2):
                            for y0 in (0, 9, 18):
                                c2pt[m, y0] = psA.tile([128, 512], F32, tag="ps",
                                                       name=f"c2pt{m}_{y0}")
                        for sh in range(25):
                            r, s = divmod(sh, 5)
                            for y0 in (0, 9, 18):
                                rhs = mkap(c2in, (y0 + r) * 32 + s,
                                           [[992, 2], [32, 9], [1, 28]])
                                for m in range(2):
                                    nc.tensor.matmul(
                                        c2pt[m, y0][:, :504],
                                        w2sb[:, sh * 256 + m * 128:
                                             sh * 256 + (m + 1) * 128],
                                        rhs, start=(sh == 0), stop=(sh == 24))
                        for m in range(2):
                            co = act.tile([128, 2 * 27 * 28], BF16, tag="c2out",
                                          bufs=2)
                            for y0 in (0, 9, 18):
                                pt = c2pt[m, y0]
                                nc.scalar.activation(
                                    mkap(co, y0 * 28, [[756, 2], [28, 9], [1, 28]]),
                                    mkap(pt, 0, [[252, 2], [28, 9], [1, 28]]),
                                    AF.Relu,
                                    bias=sc_sb[:, 2 * (SC2 + m) + 1:2 * (SC2 + m) + 2],
                                    scale=sc_sb[:, 2 * (SC2 + m):2 * (SC2 + m) + 1])
                            p2 = act.tile([128, 2 * 27 * 13], BF16, tag="p2", bufs=2)
                            max3(mkap(p2, 0, [[351, 2], [13, 27], [1, 13]]),
                                 lambda o: mkap(co, o, [[756, 2], [28, 27], [2, 13]]),
                                 1)
                            c3wv = mkap(c3in[g][m], 17, [[240, 2], [1, 13], [16, 13]])
                            nc.vector.tensor_max(
                                c3wv, mkap(p2, 0, [[351, 2], [1, 13], [26, 13]]),
                                mkap(p2, 13, [[351, 2], [1, 13], [26, 13]]))
                            nc.vector.tensor_max(
                                c3wv, c3wv, mkap(p2, 26, [[351, 2], [1, 13], [26, 13]]))

                    for g in range(4):
                        conv1_pool1(2 * g)
                        conv1_pool1(2 * g + 1)
                        if g >= 1:
                            conv2_pool2(g - 1)
                    conv2_pool2(3)

                # ---- layer-wise conv3/4/5 over the 4 image-pairs (bf16) ----
                def convq(cins, w_d, sct, nkb, M, write_fn, post_m=None):
                    nt = 9 * nkb
                    for m in range(M // 128):
                        pts = [psA.tile([128, 512], F32, tag="ps",
                                        name=f"ptq{m}_{g}") for g in range(4)]
                        wqt = wq.tile([128, nt * 128], BF16, tag="wq", bufs=2)
                        nc.scalar.dma_start(
                            out=wqt.rearrange("p (i f) -> p i f", i=nt),
                            in_=bass.AP(tensor=w_d.tensor,
                                        offset=m * nt * 128 * 128,
                                        ap=[[128, 128], [128 * 128, nt], [1, 128]]))
                        for sh in range(9):
                            r, s = divmod(sh, 3)
                            for kb in range(nkb):
                                wt = wqt[:, (sh * nkb + kb) * 128:
                                         (sh * nkb + kb + 1) * 128]
                                first = (sh == 0 and kb == 0)
                                last = (sh == 8 and kb == nkb - 1)
                                for g in range(4):
                                    rhs = mkap(cins[g][kb], r * 16 + s,
                                               [[240, 2], [16, 13], [1, 14]])
                                    nc.tensor.matmul(
                                        pts[g][:, :364], wt, rhs,
                                        start=first, stop=last)
                        for g in range(4):
                            write_fn(g, m, pts[g])
                        if post_m is not None:
                            post_m(m)

                def write_pad(nxt, sct):
                    def fn(g, m, pt):
                        nc.scalar.activation(
                            mkap(nxt[g][m], 17, [[240, 2], [16, 13], [1, 13]]),
                            mkap(pt, 0, [[182, 2], [14, 13], [1, 13]]),
                            AF.Relu,
                            bias=sc_sb[:, 2 * (sct + m) + 1:2 * (sct + m) + 2],
                            scale=sc_sb[:, 2 * (sct + m):2 * (sct + m) + 1])
                    return fn

                c4in = {g: [pad_tile(f"c4in{kb}_{g}") for kb in range(3)]
                        for g in range(4)}
                convq(c3in, w3c_d, SC3, 2, 384, write_pad(c4in, SC3))
                # c5in kb 0/1 alias c3in's buffers (dead after conv3)
                c5in = {g: [pad_tile(f"c3in{kb}_{g}") for kb in range(2)]
                        + [pad_tile(f"c5x_{g}")] for g in range(4)}
                convq(c4in, w4c_d, SC4, 3, 384, write_pad(c5in, SC4))

                def write_c5(g, m, pt):
                    c5o = p345.tile([128, 2 * 169], BF16, tag="c5out", bufs=4)
                    nc.scalar.activation(
                        mkap(c5o, 0, [[169, 2], [13, 13], [1, 13]]),
                        mkap(pt, 0, [[182, 2], [14, 13], [1, 13]]),
                        AF.Relu,
                        bias=sc_sb[:, 2 * (SC5 + m) + 1:2 * (SC5 + m) + 2],
                        scale=sc_sb[:, 2 * (SC5 + m):2 * (SC5 + m) + 1])
                    p3a = p345.tile([128, 2 * 13 * 6], BF16, tag="p3a", bufs=2)
                    max3(mkap(p3a, 0, [[78, 2], [6, 13], [1, 6]]),
                         lambda o: mkap(c5o, o, [[169, 2], [13, 13], [2, 6]]), 1)
                    hst = p345.tile([128, 2 * 36], BF16, tag="hst", bufs=2)
                    max3(mkap(hst, 0, [[36, 2], [1, 6], [6, 6]]),
                         lambda o: mkap(p3a, o, [[78, 2], [1, 6], [12, 6]]), 6)
                    nc.sync.dma_start(
                        out=rawap(h_loc, (2 * g) * 9216 + m * 4608,
                                  [[36, 128], [9216, 2], [1, 36]]),
                        in_=hst.rearrange("p (i f) -> p i f", i=2))

                def h_gather(m):
                    if m == 1:
                        nc.gpsimd.collective_compute(
                            "AllGather", mybir.AluOpType.bypass,
                            replica_groups=[list(range(NCORES))],
                            ins=[h_loc[:, :]], outs=[h_all[:, :]])

                convq(c5in, w5c_d, SC5, 3, 256, write_c5, post_m=h_gather)

            # ============== FC stack ==============
            # lhsT slices are 65 wide (out partitions 65): a 64-col lhsT drops
            # the PE to its (128,64) tile mode at half streaming rate; row 64
            # of each PSUM out is garbage and ignored.
            with tc.tile_pool(name="fcw", bufs=1) as fcw, \
                 tc.tile_pool(name="fca", bufs=1) as fca, \
                 tc.tile_pool(name="psm", bufs=1, space="PSUM") as psm:

                # fw1 second half + fw2/fw3 streamed
                fw1b = fcw.tile([128, 36 * 512], BF16, name="fw1b")
                for gi in range(3):
                    nc.scalar.dma_start(
                        out=fw1b[:, gi * 12 * 512:(gi + 1) * 12 * 512]
                            .rearrange("p (i f) -> p i f", i=12),
                        in_=bass.AP(tensor=fw1_d.tensor,
                                    offset=(36 + gi * 12) * 128 * 512,
                                    ap=[[512, 128], [128 * 512, 12], [1, 512]]))
                fw2sb = fcw.tile([128, 32 * 512], BF16, name="fw2sb")
                for gi in range(2):
                    nc.scalar.dma_start(
                        out=fw2sb[:, gi * 16 * 512:(gi + 1) * 16 * 512]
                            .rearrange("p (i f) -> p i f", i=16),
                        in_=bass.AP(tensor=fw2_d.tensor,
                                    offset=gi * 16 * 128 * 512,
                                    ap=[[512, 128], [128 * 512, 16], [1, 512]]))
                fw3sb = fcw.tile([128, 32 * 126], BF16, name="fw3sb")
                nc.scalar.dma_start(
                    out=fw3sb.rearrange("p (i f) -> p i f", i=32),
                    in_=bass.AP(tensor=fw3_d.tensor, offset=0,
                                ap=[[126, 128], [128 * 126, 32], [1, 126]]))

                # h -> h_T via XBAR DMA transpose (straight from DRAM);
                # per-m so fc1's first half starts after the first AllGather
                h_T = fca.tile([128, 72 * 64 + 64], BF16, name="h_T")
                nc.sync.dma_start(
                    out=h_T[:, :4608].rearrange("p (t f) -> p t f", t=72),
                    in_=h_all[:, :], transpose=True)

                # fc1: 4-bank PSUM rotation deepens the accumulate pipeline
                pms = [psm.tile([65, 512], F32, tag=f"pm{j}", name=f"pm{j}")
                       for j in range(4)]
                for kt in range(72):
                    wsrc = fw1a if kt < 36 else fw1b
                    wt = wsrc[:, (kt % 36) * 512:(kt % 36 + 1) * 512]
                    nc.tensor.matmul(pms[kt % 4],
                                     h_T[:, kt * 64:kt * 64 + 65], wt,
                                     start=(kt < 4), stop=(kt >= 68))

                def fc_epilogue(pms_, fbr, scf_col, n, name):
                    yc = fca.tile([64, n], F32, name=f"{name}_c")
                    nc.scalar.activation(yc, pms_[0][:64], AF.Copy)
                    ys = fca.tile([64, n], F32, name=f"{name}_s")
                    nc.vector.tensor_add(ys, yc, pms_[1][:64])
                    nc.vector.tensor_add(ys, ys, pms_[2][:64])
                    nc.vector.tensor_add(ys, ys, pms_[3][:64])
                    yf = fca.tile([64, n], F32, name=f"{name}_f")
                    nc.vector.scalar_tensor_tensor(
                        out=yf, in0=ys, scalar=sc_sb[:64, scf_col:scf_col + 1],
                        in1=fbr,
                        op0=mybir.AluOpType.mult, op1=mybir.AluOpType.add)
                    yr = fca.tile([64, n], BF16, name=f"{name}_r")
                    nc.scalar.activation(yr, yf, AF.Relu)
                    return yr

                y1r = fc_epilogue(pms, fb1r, 2 * SCF, 512, "y1")
                # ship y1 transposed: local XBAR transpose -> DRAM -> AllGather
                y1t = fca.tile([128, 4 * 64], BF16, name="y1t")
                nc.sync.dma_start(
                    out=y1t.rearrange("p (t f) -> p t f", t=4),
                    in_=y1r, transpose=True)
                nc.sync.dma_start(
                    out=rawap(y1loc, 0, [[64, 128], [128 * 64, 4], [1, 64]]),
                    in_=y1t.rearrange("p (i f) -> p i f", i=4))
                nc.gpsimd.collective_compute(
                    "AllGather", mybir.AluOpType.bypass,
                    replica_groups=[list(range(NCORES))],
                    ins=[y1loc[:, :]], outs=[y1all[:, :]])
                y1sb = fca.tile([128, 32 * 64 + 64], BF16, name="y1sb")
                nc.sync.dma_start(
                    out=y1sb[:, :2048].rearrange("p (i f) -> p i f", i=32),
                    in_=rawap(y1all, 0, [[64, 128], [128 * 64, 32], [1, 64]]))

                # fc2
                for t in range(32):
                    nc.tensor.matmul(pms[t % 4],
                                     y1sb[:, t * 64:t * 64 + 65],
                                     fw2sb[:, t * 512:(t + 1) * 512],
                                     start=(t < 4), stop=(t >= 28))
                y2r = fc_epilogue(pms, fb2r, 2 * SCF + 1, 512, "y2")

                # ship y2 transposed and gather, then fc3
                y2t = fca.tile([128, 4 * 64], BF16, name="y2t")
                nc.sync.dma_start(
                    out=y2t.rearrange("p (t f) -> p t f", t=4),
                    in_=y2r, transpose=True)
                nc.sync.dma_start(
                    out=rawap(y2loc, 0, [[64, 128], [128 * 64, 4], [1, 64]]),
                    in_=y2t.rearrange("p (i f) -> p i f", i=4))
                nc.gpsimd.collective_compute(
                    "AllGather", mybir.AluOpType.bypass,
                    replica_groups=[list(range(NCORES))],
                    ins=[y2loc[:, :]], outs=[y2all[:, :]])
                y2sb = fca.tile([128, 32 * 64 + 64], BF16, name="y2sb")
                nc.sync.dma_start(
                    out=y2sb[:, :2048].rearrange("p (i f) -> p i f", i=32),
                    in_=rawap(y2all, 0, [[64, 128], [128 * 64, 32], [1, 64]]))
                pm3A = psm.tile([65, 126], F32, tag="pm3A", name="pm3A")
                pm3B = psm.tile([65, 126], F32, tag="pm3B", name="pm3B")
                for t in range(32):
                    nc.tensor.matmul(pm3A if t % 2 == 0 else pm3B,
                                     y2sb[:, t * 64:t * 64 + 65],
                                     fw3sb[:, t * 126:(t + 1) * 126],
                                     start=(t < 2), stop=(t >= 30))
                o3c = fca.tile([64, 126], F32, name="o3c")
                nc.scalar.activation(o3c, pm3A[:64], AF.Copy)
                osb = fca.tile([64, 126], F32, name="osb")
                nc.vector.tensor_add(osb, o3c, pm3B[:64])
                nc.vector.tensor_add(osb, osb, fb3r)
                nc.sync.dma_start(out=out_d, in_=osb)

    nc.compile()
    return nc


def _get_program():
    if "nc" not in _PROG_CACHE:
        _PROG_CACHE["nc"] = _build_program()
    return _PROG_CACHE["nc"]


def _host_prep(inputs):
    eps = 1e-5
    f32 = np.float32
    bf16 = ml_dtypes.bfloat16

    def inv(g, v):
        return (g / np.sqrt(v + eps)).astype(f32)

    def rms(w):
        return np.sqrt(np.mean(w.astype(np.float64) ** 2)).astype(f32)

    x = inputs["x"]
    w1, b1 = inputs["w1"], inputs["b1"]
    inv1 = inv(inputs["g1"], inputs["v1"])
    w1f = (w1 * inv1[:, None, None, None]).astype(f32)
    b1f = (b1 * inv1 + inputs["be1"] - inputs["m1"] * inv1).astype(f32)

    # conv1 im2col: [B, 3, 121, 3072] bf16 (padded cols)
    xp = np.pad(x, ((0, 0), (0, 0), (2, 2), (2, 2)))
    s = xp.strides
    win = np.lib.stride_tricks.as_strided(
        xp, shape=(B, 3, 11, 11, 55, 55),
        strides=(s[0], s[1], s[2], s[3], 4 * s[2], 4 * s[3]))
    xcol = np.zeros((B, 3, 121, 3072), bf16)
    xcol[:, :, :, :3025] = win.reshape(B, 3, 121, 3025).astype(bf16)
    w1c = np.ascontiguousarray(
        w1f.reshape(96, 3, 121).transpose(1, 2, 0)).astype(bf16)  # [3,121,96]

    inv2 = inv(inputs["g2"], inputs["v2"])
    E2 = rms(inputs["w2"])
    sgn2 = np.sign(inputs["w2"]).astype(f32)  # [256, 96, 5, 5]
    w2c = np.ascontiguousarray(
        sgn2.reshape(256, 96, 25).transpose(2, 1, 0)).astype(f32)  # [25,96,256]

    def conv_sgn(w, nkb, M):
        # packed [M//128, 9*nkb, 128, 128]: one contiguous block per m-tile
        sgn = np.sign(w).astype(f32)  # [M, K, 3, 3]
        K = sgn.shape[1]
        out = np.zeros((M // 128, 9 * nkb, 128, 128), bf16)
        for r in range(3):
            for s_ in range(3):
                blk = sgn[:, :, r, s_].T  # [K, M]
                for kb in range(nkb):
                    kk = min(128, K - kb * 128)
                    for m in range(M // 128):
                        out[m, (r * 3 + s_) * nkb + kb, :kk] = \
                            blk[kb * 128:kb * 128 + kk,
                                m * 128:(m + 1) * 128].astype(bf16)
        return out

    E3, E4, E5 = rms(inputs["w3"]), rms(inputs["w4"]), rms(inputs["w5"])
    w3c = conv_sgn(inputs["w3"], 2, 384)
    w4c = conv_sgn(inputs["w4"], 3, 384)
    w5c = conv_sgn(inputs["w5"], 3, 256)

    Ef1, Ef2 = rms(inputs["fw1"]), rms(inputs["fw2"])
    sgnf1 = np.sign(inputs["fw1"]).astype(bf16)
    sgnf2 = np.sign(inputs["fw2"]).astype(bf16)

    # sc_all [128, 24]: col pair 2t:2t+2 = (scale, bias) for tile t
    sc = np.zeros((128, 24), f32)
    b2f = (inputs["b2"] * inv2 + inputs["be2"] - inputs["m2"] * inv2).astype(f32)
    sc[:96, 0] = 1.0
    sc[:96, 1] = b1f
    for m in range(2):
        sc[:, 2 * (1 + m)] = E2 * inv2[m * 128:(m + 1) * 128]
        sc[:, 2 * (1 + m) + 1] = b2f[m * 128:(m + 1) * 128]
    for m in range(3):
        sc[:, 2 * (3 + m)] = E3
        sc[:, 2 * (3 + m) + 1] = inputs["b3"][m * 128:(m + 1) * 128]
        sc[:, 2 * (6 + m)] = E4
        sc[:, 2 * (6 + m) + 1] = inputs["b4"][m * 128:(m + 1) * 128]
    for m in range(2):
        sc[:, 2 * (9 + m)] = E5
        sc[:, 2 * (9 + m) + 1] = inputs["b5"][m * 128:(m + 1) * 128]
    sc[:, 22] = Ef1
    sc[:, 23] = Ef2

    shared = dict(w1c=w1c, scall=sc, w2c=w2c, w3c=w3c, w4c=w4c, w5c=w5c)
    in_maps = []
    for c in range(NCORES):
        m = dict(shared)
        m["xcol"] = np.ascontiguousarray(xcol[c * BPC:(c + 1) * BPC])
        m["fw1t"] = np.ascontiguousarray(sgnf1[c * 512:(c + 1) * 512].T)
        m["fb1r"] = inputs["fb1"][c * 512:(c + 1) * 512].astype(f32).reshape(1, 512)
        m["fw2t"] = np.ascontiguousarray(sgnf2[c * 512:(c + 1) * 512].T)
        m["fb2r"] = inputs["fb2"][c * 512:(c + 1) * 512].astype(f32).reshape(1, 512)
        fw3s = np.zeros((4096, 126), bf16)
        fw3s[:, :125] = inputs["fw3"][c * 125:(c + 1) * 125].T.astype(bf16)
        m["fw3t"] = fw3s
        fb3s = np.zeros((1, 126), f32)
        fb3s[0, :125] = inputs["fb3"][c * 125:(c + 1) * 125]
        m["fb3r"] = fb3s
        in_maps.append(m)
    return in_maps


def kernel(**inputs):
    global LAST_EXEC_NS, LAST_RESULTS
    from concourse import bass_utils

    nc = _get_program()
    in_maps = _host_prep(inputs)
    trace = os.environ.get("BASS_KERNEL_TRACE", "0") == "1"
    res = bass_utils.run_bass_kernel_spmd(
        nc, in_maps, core_ids=list(range(NCORES)), trace=trace)
    LAST_EXEC_NS = res.exec_time_ns
    LAST_RESULTS = res

    out = np.zeros((B, 1000), np.float32)
    for c in range(NCORES):
        out[:, c * 125:(c + 1) * 125] = res.results[c]["out"][:, :125]
    return out
